# revision 1
# baseline (speedup 1.0000x reference)
"""Trainium2 Bass kernel: nn_CorrBlockSingleScale (RAFT single-scale correlation lookup).

reference: corr[b,n] = fmap1[b,:,n] . fmap2[b,:,m] / 16 as a [HW, H, W] volume;
out[b, k1*9+k2, h, w] = bilinear(corr[b,(h,w)], x=cx+k1-4, y=cy+k2-4), zeros padding.

Sharding: data-parallel over the B*H*W = 8192 pixel axis; core c handles batch
c//4, pixels (c%4)*1024 ... +1024 (8 blocks of 128 pixels). No cross-core comms.

Per 128-pixel block on each core:
  PE      : C[128pix, 4096] = f1_blk^T @ f2 (fp32r, N=512 chunks, K=256)
  DVE/ACT : PSUM -> SBUF copy (split engines)
  DMA     : C -> DRAM table [pix, 65, 64] (row 64 stays zero; y-OOB taps index it)
  gather  : dma_gather 10 map rows/pixel (256B rows) -> G[128, 10, 64]
  DVE/ACT : y-lerp (per-partition u), 6-stage binary select ladder for the
            per-pixel x-window shift (bits of floor(cx) as masks), x-lerp (v)
  DMA     : O[128, 81] -> out
Host: index/weight precompute from coords, final reshape/transpose.
"""

import numpy as np

import concourse.bass as bass
import concourse.mybir as mybir
import concourse.tile as tile
from concourse import bacc, library_config
from concourse.bass_utils import run_bass_kernel_spmd

F32 = mybir.dt.float32
F32R = mybir.dt.float32r
I16 = mybir.dt.int16
COPY = mybir.ActivationFunctionType.Copy
MULT = mybir.AluOpType.mult

NCORES = 8
NPC = 1024          # pixels per core
NBLK = 8            # blocks of 128 pixels per core
ROWS = 65           # 64 data rows + 1 zero row per pixel in the DRAM table
WPAD = 76           # padded row width for the shift ladder (4 + 64 + 5 + align)
STAGES = [(32, 41), (16, 25), (8, 17), (4, 13), (2, 11), (1, 10)]

_NC = None


def _s(st, b, j):
    """[128,1] per-partition scalar view of scal tile column j for block b."""
    return st[:, b : b + 1, j : j + 1].rearrange("p a c -> p (a c)")


def _build_kernel(tc, out, corr, f1, f2, idx, scal):
    nc = tc.nc
    import contextlib

    with contextlib.ExitStack() as ctx:
        const = ctx.enter_context(tc.tile_pool(name="const", bufs=1))
        work = ctx.enter_context(tc.tile_pool(name="work", bufs=3))
        cpool = ctx.enter_context(tc.tile_pool(name="cpool", bufs=2))
        psum = ctx.enter_context(tc.tile_pool(name="psum", bufs=8, space="PSUM"))

        nc.gpsimd.load_library(library_config.mlp)

        f1t = const.tile([128, 2, NPC], F32R)
        nc.sync.dma_start(f1t[:], f1[:])
        f2t = const.tile([128, 2, 4096], F32R)
        nc.sync.dma_start(f2t[:], f2[:])
        idxt = const.tile([128, NBLK, 80], I16)
        nc.sync.dma_start(idxt[:], idx[:])
        st = const.tile([128, NBLK, 16], F32)
        nc.sync.dma_start(st[:], scal[:])

        # zero row 64 of every pixel's table (y-OOB taps gather from it)
        zt = const.tile([128, 64], F32)
        nc.vector.memset(zt[:], 0.0)
        for b in range(NBLK):
            nc.sync.dma_start(
                corr[b * 128 : (b + 1) * 128, 64:65, :],
                zt[:].rearrange("p (a c) -> p a c", a=1),
            )

        # persistent padded tiles; pad columns stay zero across blocks
        xps = [
            const.tile([128, 9, WPAD], F32, name=f"xp{i}", tag=f"xp{i}")
            for i in range(2)
        ]
        nc.vector.memset(xps[0][:], 0.0)
        nc.vector.memset(xps[1][:], 0.0)

        for b in range(NBLK):
            pix = slice(b * 128, (b + 1) * 128)
            # ---- matmul: C = f1_blk^T @ f2  (K=256 as 2x128, N=4096 as 8x512)
            lhs = [
                f1t[:, k : k + 1, b * 128 : (b + 1) * 128]
                .rearrange("p a c -> p (a c)")
                for k in range(2)
            ]
            pts = []
            for n in range(8):
                pt = psum.tile([128, 512], F32, tag="ps")
                pts.append(pt)
                for k in range(2):
                    nc.tensor.matmul(
                        pt[:],
                        lhsT=lhs[k],
                        rhs=f2t[:, k : k + 1, n * 512 : (n + 1) * 512]
                        .rearrange("p a c -> p (a c)"),
                        start=(k == 0),
                        stop=(k == 1),
                    )
            ct = cpool.tile([128, 4096], F32, tag="C")
            for n in range(8):
                dst = ct[:, n * 512 : (n + 1) * 512]
                if n % 2 == 0:
                    nc.vector.tensor_copy(dst, pts[n][:])
                else:
                    nc.scalar.copy(dst, pts[n][:])
            # ---- write corr block to the DRAM table (rows 0..63)
            nc.sync.dma_start(
                corr[pix, 0:64, :], ct[:].rearrange("p (a c) -> p a c", a=64)
            )
            # ---- gather 10 rows per pixel
            gt = work.tile([128, 10, 64], F32, tag="G")
            table = corr[pix].rearrange("a b c -> (a b) c")
            nc.gpsimd.dma_gather(
                gt[:],
                table,
                idxt[:, b : b + 1, :].rearrange("p a c -> p (a c)"),
                1280,
                1280,
                64,
                single_packet=False,
            )
            # ---- y-lerp into the padded tile: xp[:,:,4:68] = (1-u)G0 + u*G1
            xp = xps[b % 2]
            t0 = work.tile([128, 9, 64], F32, tag="T0")
            nc.scalar.activation(t0[:], gt[:, 0:9, :], COPY, scale=_s(st, b, 0))
            nc.vector.tensor_scalar(
                xp[:, :, 4:68], gt[:, 1:10, :], _s(st, b, 1), None, MULT
            )
            nc.vector.tensor_add(xp[:, :, 4:68], xp[:, :, 4:68], t0[:])
            # ---- 6-stage binary select ladder: shift by sx = floor(cx) in [0,63]
            cur = xp[:, :, 0:73]
            for k, (sh, wn) in enumerate(STAGES):
                nxt = work.tile([128, 9, wn], F32, tag=f"L{k}")
                mask = (
                    _s(st, b, 4 + k)
                    .bitcast(mybir.dt.int32)
                    .to_broadcast([128, 9, wn])
                )
                nc.scalar.copy(nxt[:], cur[:, :, 0:wn])
                nc.vector.copy_predicated(nxt[:], mask, cur[:, :, sh : sh + wn])
                cur = nxt
            # ---- x-lerp: O = (1-v)X[0:9] + v*X[1:10]
            t1 = work.tile([128, 9, 9], F32, tag="T1")
            nc.scalar.activation(t1[:], cur[:, :, 0:9], COPY, scale=_s(st, b, 2))
            ot = work.tile([128, 9, 9], F32, tag="OT")
            nc.vector.tensor_scalar(ot[:], cur[:, :, 1:10], _s(st, b, 3), None, MULT)
            nc.vector.tensor_add(ot[:], ot[:], t1[:])
            nc.sync.dma_start(out[pix, :], ot[:].rearrange("p a c -> p (a c)"))


def _build():
    nc = bacc.Bacc("TRN2", target_bir_lowering=False, debug=False)
    f1 = nc.dram_tensor("f1", [128, 2, NPC], F32R, kind="ExternalInput").ap()
    f2 = nc.dram_tensor("f2", [128, 2, 4096], F32R, kind="ExternalInput").ap()
    idx = nc.dram_tensor("idx", [128, NBLK, 80], I16, kind="ExternalInput").ap()
    scal = nc.dram_tensor("scal", [128, NBLK, 16], F32, kind="ExternalInput").ap()
    out = nc.dram_tensor("out", [NPC, 81], F32, kind="ExternalOutput").ap()
    corr = nc.dram_tensor("corr", [NPC, ROWS, 64], F32, kind="Internal").ap()
    with tile.TileContext(nc) as tc:
        _build_kernel(tc, out, corr, f1, f2, idx, scal)
    nc.compile()
    return nc


def get_nc():
    global _NC
    if _NC is None:
        _NC = _build()
    return _NC


def host_prep(fmap1, fmap2, coords, radius):
    """Per-core input maps. All coord-derived indices/weights computed here."""
    B, D, H, W = fmap1.shape
    assert (B, D, H, W) == (2, 256, 64, 64) and int(radius) == 4
    f1 = (fmap1.reshape(B, D, H * W) / np.float32(16.0)).astype(np.float32)
    f2 = fmap2.reshape(B, D, H * W).astype(np.float32)
    cx = coords[:, 0].reshape(B, H * W).astype(np.float32)
    cy = coords[:, 1].reshape(B, H * W).astype(np.float32)

    in_maps = []
    for c in range(NCORES):
        b, ps = c // 4, (c % 4) * NPC
        f1c = np.ascontiguousarray(
            f1[b, :, ps : ps + NPC].reshape(2, 128, NPC).transpose(1, 0, 2)
        )
        f2c = np.ascontiguousarray(f2[b].reshape(2, 128, 4096).transpose(1, 0, 2))
        ccx = cx[b, ps : ps + NPC]
        ccy = cy[b, ps : ps + NPC]
        x0 = np.floor(ccx)
        y0 = np.floor(ccy)
        v = ccx - x0  # frac x
        u = ccy - y0  # frac y
        sx = x0.astype(np.int64)  # in [0, 63]
        y0i = y0.astype(np.int64)

        # gather indices: [8 blocks][1280] -> wrapped [128, 80] int16
        idxc = np.zeros((128, NBLK, 80), np.int16)
        scalc = np.zeros((128, NBLK, 16), np.float32)
        for blk in range(NBLK):
            p = np.arange(128)
            gp = blk * 128 + p
            rows = y0i[gp][None, :] - 4 + np.arange(10)[:, None]  # [10, 128]
            rows = np.where((rows < 0) | (rows > 63), 64, rows)
            ilist = (p[None, :] * ROWS + rows).reshape(1280).astype(np.int16)
            idxc[:, blk, :] = np.tile(ilist.reshape(80, 16).T, (8, 1))
            scalc[:, blk, 0] = 1.0 - u[gp]
            scalc[:, blk, 1] = u[gp]
            scalc[:, blk, 2] = 1.0 - v[gp]
            scalc[:, blk, 3] = v[gp]
            for k, (sh, _) in enumerate(STAGES):
                scalc[:, blk, 4 + k] = ((sx[gp] // sh) % 2).astype(np.float32)
        in_maps.append(
            {
                "f1": f1c,
                "f2": f2c,
                "idx": np.ascontiguousarray(idxc),
                "scal": np.ascontiguousarray(scalc),
            }
        )
    return in_maps


def assemble(outs):
    """8x [1024, 81] (k2-major within 81) -> [2, 81, 64, 64] with k = k1*9+k2."""
    o = np.stack(outs).reshape(2, 4096, 81).reshape(2, 64, 64, 9, 9)
    return np.ascontiguousarray(
        o.transpose(0, 4, 3, 1, 2).reshape(2, 81, 64, 64)
    ).astype(np.float32)


def kernel(**inputs):
    fmap1 = np.asarray(inputs["fmap1"], np.float32)
    fmap2 = np.asarray(inputs["fmap2"], np.float32)
    coords = np.asarray(inputs["coords"], np.float32)
    radius = int(np.asarray(inputs["radius"]))
    in_maps = host_prep(fmap1, fmap2, coords, radius)
    nc = get_nc()
    res = run_bass_kernel_spmd(nc, in_maps, core_ids=list(range(NCORES)))
    return assemble([r["out"] for r in res.results])



# revision 13
# speedup vs baseline: 2.0084x; 2.0084x over previous
"""Trainium2 Bass kernel: nn_CorrBlockSingleScale (RAFT single-scale corr lookup).

reference: corr[n, m] = fmap1[b,:,n] . fmap2[b,:,m] / 16 over m = (ym, xm) in a
64x64 grid; out[b, k1*9+k2, h, w] = bilinear(corr[(h,w)], x=cx+k2-4, y=cy+k1-4),
zeros padding (grid_sample align_corners=True, padding_mode='zeros').

Sharding: data-parallel over the B*H*W = 8192 pixel axis; core c handles batch
c//4, pixels (c%4)*1024 .. +1024.  No cross-core comms.

Key idea: pixels are HOST-SORTED by y0 = floor(cy).  A block of 128 sorted
pixels spans only ~9 distinct y0 values, so its correlation slice needs just
RP ~ 20 rows of the 64-row (ym) grid:

  PE    : C[128pix, RP*64] = f1_blk^T @ f2[:, tbase*64 : (tbase+RP)*64]  (bf16)
  DVE/..: PSUM -> SBUF ct (bf16), memset OOB-ym edge rows to zero
  Pool  : ap_gather (SBUF->SBUF, per-16-partition-group window start) pulls a
          16-row window covering every pixel's 10-row y-window
  y-lerp: weighted (OMAX+2)-tap combine with per-partition scalar weights
          (within-group offset o in [0,OMAX] folded into host weights)
  x     : 5-stage binary shift ladder (int32-pair tricks; shifts 32..2) + a
          3-tap x-lerp (final odd shift folded into per-partition weights)
Host: sort/permute pixels, compute indices/weights, inverse-permute output.
"""

import numpy as np

import concourse.bass as bass
import concourse.mybir as mybir
import concourse.tile as tile
from concourse import bacc, library_config
from concourse.bass_utils import run_bass_kernel_spmd

F32 = mybir.dt.float32
BF16 = mybir.dt.bfloat16
I32 = mybir.dt.int32
I16 = mybir.dt.int16
COPY = mybir.ActivationFunctionType.Copy
MULT = mybir.AluOpType.mult
ADD = mybir.AluOpType.add

NCORES = 8
NPC = 1024          # pixels per core
NBLK = 8            # blocks of 128 sorted pixels
JUNK = 6            # extra ct rows so a 16-row gather may overrun data rows
XSH = [16, 8, 4, 2, 1]   # x ladder shifts in int32 elems (bf16: 32,16,8,4,2)

_NC_CACHE = {}


def _s(st, b, j):
    """[128,1] per-partition scalar view of scal column j for block b."""
    return st[:, b : b + 1, j : j + 1].rearrange("p a c -> p (a c)")


def _build_kernel(tc, out, f1, f2, idx, scal, tbases, RP, YT):
    nc = tc.nc
    import contextlib

    RPA = RP + JUNK
    with contextlib.ExitStack() as ctx:
        const = ctx.enter_context(tc.tile_pool(name="const", bufs=1))
        work = ctx.enter_context(tc.tile_pool(name="work", bufs=3))
        cpool = ctx.enter_context(tc.tile_pool(name="cpool", bufs=2))
        psum = ctx.enter_context(tc.tile_pool(name="psum", bufs=2, space="PSUM"))

        nc.gpsimd.load_library(library_config.ap_gather)

        f1t = const.tile([128, 2, NPC], BF16)
        nc.sync.dma_start(f1t[:], f1[:])
        f2t = const.tile([128, 2, 4096], BF16)
        for cch in range(8):
            sl = slice(cch * 512, (cch + 1) * 512)
            nc.sync.dma_start(f2t[:, :, sl], f2[:, :, sl])
        idxt = const.tile([128, NBLK], I16)
        nc.sync.dma_start(idxt[:], idx[:])
        st = const.tile([128, NBLK, 16], F32)
        nc.sync.dma_start(st[:], scal[:])

        # persistent padded x tiles; pad columns stay zero across blocks
        xps = [
            const.tile([128, 9, 74], BF16, name=f"xp{i}", tag=f"xp{i}")
            for i in range(2)
        ]
        nc.vector.memset(xps[0][:], 0.0)
        nc.vector.memset(xps[1][:], 0.0)

        for b in range(NBLK):
            tbase = tbases[b]
            vlo = max(0, -tbase)           # first valid ct row
            vhi = min(RP, 64 - tbase)      # one past last valid ct row
            nval = (vhi - vlo) * 64
            c0 = (tbase + vlo) * 64        # f2 column of first valid row

            # ---- matmul: C_valid = f1_blk^T @ f2[:, c0 : c0+nval]  (k-outer)
            ps = psum.tile([128, RP * 64], F32, tag="ps")
            chunks = []
            o = 0
            while o < nval:
                n = min(512, nval - o)
                chunks.append((o, n))
                o += n
            for k in range(2):
                lhs = f1t[:, k : k + 1, b * 128 : (b + 1) * 128].rearrange(
                    "p a c -> p (a c)"
                )
                for (o, n) in chunks:
                    nc.tensor.matmul(
                        ps[:, o : o + n],
                        lhsT=lhs,
                        rhs=f2t[:, k : k + 1, c0 + o : c0 + o + n].rearrange(
                            "p a c -> p (a c)"
                        ),
                        start=(k == 0),
                        stop=(k == 1),
                    )

            # ---- PSUM -> ct (bf16); ct rows [vlo, vhi) hold C, edges zero
            ct = cpool.tile([128, RPA, 64], BF16, tag="C")
            ctf = ct[:].rearrange("p a c -> p (a c)")
            if b < 2:
                # junk rows (gather overrun space): zero once per pool buffer
                nc.gpsimd.memset(ct[:, RP:RPA, :], 0.0)
            if vlo > 0:
                nc.gpsimd.memset(ct[:, 0:vlo, :], 0.0)
            if vhi < RP:
                nc.gpsimd.memset(ct[:, vhi:RP, :], 0.0)
            base_e = vlo * 64
            # split copies: [0:512) DVE, [512:nval) ACT
            h = min(512, nval)
            nc.vector.tensor_copy(ctf[:, base_e : base_e + h], ps[:, 0:h])
            if nval > h:
                nc.scalar.copy(ctf[:, base_e + h : base_e + nval], ps[:, h:nval])

            # ---- SBUF gather: 16-row window per 16-partition group
            gt = work.tile([128, 16, 32], I32, tag="G")
            nc.gpsimd.ap_gather(
                gt[:],
                ct[:].bitcast(I32),
                idxt[:, b : b + 1],
                128,     # channels
                RPA,     # num_elems
                32,      # d (int32 per row)
                16,      # num_idxs
            )
            g = gt[:].bitcast(BF16)  # [128, 16, 64]

            # ---- y-lerp: xp[:, :, 4:68] = sum_t w_t * G[t : t+9]
            xp = xps[b % 2]
            t0 = work.tile([128, 9, 64], F32, tag="T0")
            nc.scalar.activation(t0[:], g[:, 0:9, :], COPY, scale=_s(st, b, 0))
            acc = t0[:]
            for t in range(1, YT):
                if t == YT - 1:
                    dst = xp[:, :, 4:68]
                else:
                    tt = work.tile([128, 9, 64], F32, name=f"T{t}", tag=f"T{t}")
                    dst = tt[:]
                nc.vector.scalar_tensor_tensor(
                    dst, g[:, t : t + 9, :], _s(st, b, t), acc, MULT, ADD
                )
                acc = dst

            # ---- x shift ladder (shift by (x0 & ~1); 5 stages)
            cur32 = xp[:].bitcast(I32)          # [128, 9, 37]
            w = 37
            for kst, sh in enumerate(XSH):
                wn = w - sh
                nxt = work.tile([128, 9, wn], I32, tag=f"L{kst}")
                mask = (
                    _s(st, b, 8 + kst)
                    .bitcast(I32)
                    .to_broadcast([128, 9, wn])
                )
                nc.gpsimd.tensor_copy(nxt[:], cur32[:, :, 0:wn])
                nc.vector.copy_predicated(nxt[:], mask, cur32[:, :, sh : sh + wn])
                cur32 = nxt[:]
                w = wn
            curb = cur32.bitcast(BF16)          # [128, 9, 12]

            # ---- x-lerp: ot = sum_t wv_t * cur[t : t+9]
            t1 = work.tile([128, 9, 9], F32, tag="X0")
            nc.scalar.activation(t1[:], curb[:, :, 0:9], COPY, scale=_s(st, b, 4))
            t2 = work.tile([128, 9, 9], F32, tag="X1")
            nc.vector.scalar_tensor_tensor(
                t2[:], curb[:, :, 1:10], _s(st, b, 5), t1[:], MULT, ADD
            )
            ot = work.tile([128, 9, 9], F32, tag="OT")
            nc.vector.scalar_tensor_tensor(
                ot[:], curb[:, :, 2:11], _s(st, b, 6), t2[:], MULT, ADD
            )
            nc.sync.dma_start(
                out[b * 128 : (b + 1) * 128, :],
                ot[:].rearrange("p a c -> p (a c)"),
            )


def _build(params):
    tbases, RP, YT = params
    nc = bacc.Bacc("TRN2", target_bir_lowering=False, debug=False)
    f1 = nc.dram_tensor("f1", [128, 2, NPC], BF16, kind="ExternalInput").ap()
    f2 = nc.dram_tensor("f2", [128, 2, 4096], BF16, kind="ExternalInput").ap()
    idx = nc.dram_tensor("idx", [128, NBLK], I16, kind="ExternalInput").ap()
    scal = nc.dram_tensor("scal", [128, NBLK, 16], F32, kind="ExternalInput").ap()
    out = nc.dram_tensor("out", [NPC, 81], F32, kind="ExternalOutput").ap()
    with tile.TileContext(nc) as tc:
        _build_kernel(tc, out, f1, f2, idx, scal, tbases, RP, YT)
    nc.compile()
    return nc


def get_nc(params):
    if params not in _NC_CACHE:
        _NC_CACHE[params] = _build(params)
    return _NC_CACHE[params]


def host_prep(fmap1, fmap2, coords, radius):
    """Per-core input maps + compile params. All coord-derived logic here."""
    import ml_dtypes

    bf16 = ml_dtypes.bfloat16
    B, D, H, W = fmap1.shape
    assert (B, D, H, W) == (2, 256, 64, 64) and int(radius) == 4
    f1 = (fmap1.reshape(B, D, H * W) / np.float32(16.0)).astype(np.float32)
    f2 = fmap2.reshape(B, D, H * W).astype(np.float32)
    cx = coords[:, 0].reshape(B, H * W).astype(np.float32)
    cy = coords[:, 1].reshape(B, H * W).astype(np.float32)

    cores = []
    for c in range(NCORES):
        b, ps = c // 4, (c % 4) * NPC
        ccx = cx[b, ps : ps + NPC]
        ccy = cy[b, ps : ps + NPC]
        y0 = np.floor(ccy).astype(np.int64)
        order = np.argsort(y0, kind="stable")
        cores.append((b, ps, ccx, ccy, y0, order))

    # uniform (cross-core) per-block table bases and row count
    tbases, rtops = [], []
    for blk in range(NBLK):
        ylo = min(
            int(co[4][co[5][blk * 128 : (blk + 1) * 128]].min()) for co in cores
        )
        yhi = max(
            int(co[4][co[5][blk * 128 : (blk + 1) * 128]].max()) for co in cores
        )
        tbases.append(ylo - 4)
        rtops.append(yhi + 6)
    RP = max(rt - tb for rt, tb in zip(rtops, tbases))

    # max within-16-group y0 offset -> number of y taps
    omax = 0
    for co in cores:
        ys = co[4][co[5]]
        for g in range(NPC // 16):
            gy = ys[g * 16 : (g + 1) * 16]
            omax = max(omax, int(gy.max() - gy.min()))
    assert omax <= 6, f"group span {omax} too large for 16-row gather"
    YT = omax + 2
    assert YT <= 4, f"YT={YT}: widen scal layout for more y taps"
    params = (tuple(tbases), RP, YT)

    in_maps = []
    for (b, ps, ccx, ccy, y0, order) in cores:
        f1c = np.ascontiguousarray(
            f1[b, :, ps : ps + NPC][:, order].reshape(2, 128, NPC).transpose(1, 0, 2)
        ).astype(bf16)
        f2c = np.ascontiguousarray(
            f2[b].reshape(2, 128, 4096).transpose(1, 0, 2)
        ).astype(bf16)

        ys = y0[order]
        us = (ccy - np.floor(ccy))[order].astype(np.float32)
        x0 = np.floor(ccx).astype(np.int64)[order]
        vs = (ccx - np.floor(ccx))[order].astype(np.float32)

        idxc = np.zeros((128, NBLK), np.int16)
        scalc = np.zeros((128, NBLK, 16), np.float32)
        for blk in range(NBLK):
            sl = slice(blk * 128, (blk + 1) * 128)
            yb, ub, xb, vb = ys[sl], us[sl], x0[sl], vs[sl]
            tb = tbases[blk]
            p = np.arange(128)
            grp = p // 16
            sg = np.array(
                [int(yb[g * 16 : (g + 1) * 16].min()) - 4 - tb for g in range(8)]
            )
            assert (sg >= 0).all() and (sg <= RP - 10).all()
            idxc[:, blk] = (sg[grp] + (p % 16)).astype(np.int16)
            o = (yb - 4 - tb) - sg[grp]          # within-group offset
            assert (o >= 0).all() and (o <= YT - 2).all()
            # y taps: weight (1-u) at tap o, u at tap o+1
            for t in range(YT):
                scalc[:, blk, t] = np.where(o == t, 1.0 - ub, 0.0) + np.where(
                    o + 1 == t, ub, 0.0
                )
            # x taps: weight (1-v) at tap s, v at tap s+1 (s = x0 & 1)
            s = (xb & 1).astype(np.int64)
            for t in range(3):
                scalc[:, blk, 4 + t] = np.where(s == t, 1.0 - vb, 0.0) + np.where(
                    s + 1 == t, vb, 0.0
                )
            # ladder: shift by (x0 >> 1) in int32 elems; stages XSH
            # binary decomposition (hsh in [0,31]; XSH = 16,8,4,2,1)
            hsh = xb >> 1
            for kst, shv in enumerate(XSH):
                scalc[:, blk, 8 + kst] = ((hsh // shv) % 2).astype(np.float32)
        in_maps.append(
            {
                "f1": f1c,
                "f2": f2c,
                "idx": np.ascontiguousarray(idxc),
                "scal": np.ascontiguousarray(scalc),
            }
        )
    return in_maps, params, [co[5] for co in cores]


def assemble(outs, orders):
    """8x [1024, 81] (sorted order; 81 = ytap*9+xtap) -> [2, 81, 64, 64].

    Reference quirk (RAFT): output tap index k = k1*9+k2 where k1 is the X
    offset and k2 the Y offset, so transpose x-tap in front.
    """
    full = np.zeros((NCORES, NPC, 81), np.float32)
    for i, (o, orderv) in enumerate(zip(outs, orders)):
        full[i][orderv] = o
    o = full.reshape(2, 4096, 81).reshape(2, 64, 64, 9, 9)
    return np.ascontiguousarray(
        o.transpose(0, 4, 3, 1, 2).reshape(2, 81, 64, 64)
    ).astype(np.float32)


def kernel(**inputs):
    fmap1 = np.asarray(inputs["fmap1"], np.float32)
    fmap2 = np.asarray(inputs["fmap2"], np.float32)
    coords = np.asarray(inputs["coords"], np.float32)
    radius = int(np.asarray(inputs["radius"]))
    in_maps, params, orders = host_prep(fmap1, fmap2, coords, radius)
    nc = get_nc(params)
    res = run_bass_kernel_spmd(nc, in_maps, core_ids=list(range(NCORES)))
    return assemble([r["out"] for r in res.results], orders)


# revision 14
# speedup vs baseline: 2.6006x; 1.2948x over previous
"""Trainium2 Bass kernel: nn_CorrBlockSingleScale (RAFT single-scale corr lookup).

reference: corr[n, m] = fmap1[b,:,n] . fmap2[b,:,m] / 16 over m = (ym, xm) in a
64x64 grid; out[b, k1*9+k2, h, w] = bilinear(corr[(h,w)], x=cx+d[k1], y=cy+d[k2]),
zeros padding (grid_sample align_corners=True, padding_mode='zeros').

Sharding: data-parallel over the B*H*W = 8192 pixel axis; core c handles batch
c//4, pixels (c%4)*1024 .. +1024.  No cross-core comms.

Pixels are HOST-SORTED by y0 = floor(cy).  A block of 128 sorted pixels spans
~9 distinct y0 values, so its correlation slice needs only ~20 rows of the
64-row (ym) grid:

  PE    : C[128pix, rows*64] = f1_blk^T @ f2[:, ...]        (bf16, k-outer)
  DVE/..: PSUM -> ct rows (bf16), written into x-padded 74-wide rows
  Pool  : ap_gather (SBUF, per-16-partition-group start) pulls 16 padded rows
          covering every pixel's 10-row y-window
  DVE   : 5-stage in-place binary x-shift ladder (int32 pairs, shifts 32..2)
  DVE/ACT: 3-tap x-lerp (odd shift folded into weights), then 4-tap y-lerp
          (within-group y offset folded into weights); per-partition scalars
Host: sort pixels, compute indices/weights, inverse-permute output.
"""

import numpy as np

import concourse.bass as bass
import concourse.mybir as mybir
import concourse.tile as tile
from concourse import bacc, library_config
from concourse.bass_utils import run_bass_kernel_spmd

F32 = mybir.dt.float32
BF16 = mybir.dt.bfloat16
I32 = mybir.dt.int32
I16 = mybir.dt.int16
COPY = mybir.ActivationFunctionType.Copy
MULT = mybir.AluOpType.mult
ADD = mybir.AluOpType.add

NCORES = 8
NPC = 1024          # pixels per core
NBLK = 8            # blocks of 128 sorted pixels
JUNK = 6            # extra ct rows so a 16-row gather may overrun data rows
WROW = 74           # padded ct row width (4 | 64 | 6), bf16
XSH = [16, 8, 4, 2, 1]   # x ladder shifts in int32 elems (bf16: 32,16,8,4,2)

_NC_CACHE = {}


def _s(st, b, j):
    """[128,1] per-partition scalar view of scal column j for block b."""
    return st[:, b : b + 1, j : j + 1].rearrange("p a c -> p (a c)")


def _build_kernel(tc, out, f1, f2, idx, scal, tbases, rows, RP, YT):
    nc = tc.nc
    import contextlib

    RPA = RP + JUNK
    with contextlib.ExitStack() as ctx:
        const = ctx.enter_context(tc.tile_pool(name="const", bufs=1))
        work = ctx.enter_context(tc.tile_pool(name="work", bufs=3))
        cpool = ctx.enter_context(tc.tile_pool(name="cpool", bufs=2))
        psum = ctx.enter_context(tc.tile_pool(name="psum", bufs=2, space="PSUM"))

        nc.gpsimd.load_library(library_config.ap_gather)

        f1t = const.tile([128, 2, NPC], BF16)
        nc.sync.dma_start(f1t[:], f1[:])
        f2t = const.tile([128, 2, 4096], BF16)
        for cch in range(8):
            sl = slice(cch * 512, (cch + 1) * 512)
            nc.sync.dma_start(f2t[:, :, sl], f2[:, :, sl])
        idxt = const.tile([128, NBLK], I16)
        nc.sync.dma_start(idxt[:], idx[:])
        st = const.tile([128, NBLK, 16], F32)
        nc.sync.dma_start(st[:], scal[:])

        for b in range(NBLK):
            tbase = tbases[b]
            rws = rows[b]
            vlo = max(0, -tbase)            # first valid ct row
            vhi = min(rws, 64 - tbase)      # one past last valid ct row
            nval = (vhi - vlo) * 64
            c0 = (tbase + vlo) * 64         # f2 column of first valid row

            # ---- matmul: C_valid = f1_blk^T @ f2[:, c0 : c0+nval]  (k-outer)
            ps = psum.tile([128, RP * 64], F32, tag="ps")
            chunks = []
            o = 0
            while o < nval:
                n = min(512, nval - o)
                chunks.append((o, n))
                o += n
            for k in range(2):
                lhs = f1t[:, k : k + 1, b * 128 : (b + 1) * 128].rearrange(
                    "p a c -> p (a c)"
                )
                for (o, n) in chunks:
                    nc.tensor.matmul(
                        ps[:, o : o + n],
                        lhsT=lhs,
                        rhs=f2t[:, k : k + 1, c0 + o : c0 + o + n].rearrange(
                            "p a c -> p (a c)"
                        ),
                        start=(k == 0),
                        stop=(k == 1),
                    )

            # ---- PSUM -> ct (bf16) into x-padded 74-wide rows
            ct = cpool.tile([128, RPA, WROW], BF16, tag="C")
            if b < 2:
                # zero everything once per pool buffer: x pads stay zero
                # forever; junk/edge rows start zero (later blocks overwrite
                # data rows only, leaving finite values elsewhere)
                nc.gpsimd.memset(ct[:], 0.0)
            if vlo > 0:
                nc.gpsimd.memset(ct[:, 0:vlo, 4:68], 0.0)
            if vhi < rws:
                nc.gpsimd.memset(ct[:, vhi:rws, 4:68], 0.0)
            # split rows: first 8 to DVE, rest to ACT
            nr = vhi - vlo
            nd = min(8, nr)
            psv = ps[:, 0:nval].rearrange("p (r c) -> p r c", c=64)
            nc.vector.tensor_copy(ct[:, vlo : vlo + nd, 4:68], psv[:, 0:nd, :])
            if nr > nd:
                nc.scalar.copy(ct[:, vlo + nd : vhi, 4:68], psv[:, nd:nr, :])

            # ---- SBUF gather: 16 padded rows per 16-partition group
            gp = work.tile([128, 16, WROW], BF16, tag="G")
            nc.gpsimd.ap_gather(
                gp[:].bitcast(I32),
                ct[:].bitcast(I32),
                idxt[:, b : b + 1],
                128,        # channels
                RPA,        # num_elems
                WROW // 2,  # d (int32 per padded row)
                16,         # num_idxs
            )

            # ---- in-place binary x-shift ladder on rows 0..12 (int32 view)
            g32 = gp[:, 0:12, :].bitcast(I32)   # [128, 12, 37]
            w = 37
            for kst, sh in enumerate(XSH):
                wn = w - sh
                mask = (
                    _s(st, b, 8 + kst).bitcast(I32).to_broadcast([128, 12, wn])
                )
                nc.vector.copy_predicated(
                    g32[:, :, 0:wn], mask, g32[:, :, sh : sh + wn]
                )
                w = wn
            gb = g32.bitcast(BF16)              # [128, 12, 12]

            # ---- 3-tap x-lerp: xo = sum_t wv_t * gb[:, :, t:t+9]
            t1 = work.tile([128, 12, 9], BF16, tag="X0")
            nc.scalar.activation(t1[:], gb[:, :, 0:9], COPY, scale=_s(st, b, 4))
            for t in (1, 2):
                nc.vector.scalar_tensor_tensor(
                    t1[:], gb[:, :, t : t + 9], _s(st, b, 4 + t), t1[:], MULT, ADD
                )

            # ---- 4-tap y-lerp: ot = sum_t w_t * xo[:, t:t+9, :]
            ot = work.tile([128, 9, 9], BF16, tag="OT")
            nc.scalar.activation(ot[:], t1[:, 0:9, :], COPY, scale=_s(st, b, 0))
            for t in range(1, YT):
                nc.vector.scalar_tensor_tensor(
                    ot[:], t1[:, t : t + 9, :], _s(st, b, t), ot[:], MULT, ADD
                )
            nc.sync.dma_start(
                out[b * 128 : (b + 1) * 128, :],
                ot[:].rearrange("p a c -> p (a c)"),
            )


def _build(params):
    tbases, rows, RP, YT = params
    nc = bacc.Bacc("TRN2", target_bir_lowering=False, debug=False)
    f1 = nc.dram_tensor("f1", [128, 2, NPC], BF16, kind="ExternalInput").ap()
    f2 = nc.dram_tensor("f2", [128, 2, 4096], BF16, kind="ExternalInput").ap()
    idx = nc.dram_tensor("idx", [128, NBLK], I16, kind="ExternalInput").ap()
    scal = nc.dram_tensor("scal", [128, NBLK, 16], F32, kind="ExternalInput").ap()
    out = nc.dram_tensor("out", [NPC, 81], BF16, kind="ExternalOutput").ap()
    with tile.TileContext(nc) as tc:
        _build_kernel(tc, out, f1, f2, idx, scal, tbases, rows, RP, YT)
    nc.compile()
    return nc


def get_nc(params):
    if params not in _NC_CACHE:
        _NC_CACHE[params] = _build(params)
    return _NC_CACHE[params]


def host_prep(fmap1, fmap2, coords, radius):
    """Per-core input maps + compile params. All coord-derived logic here."""
    import ml_dtypes

    bf16 = ml_dtypes.bfloat16
    B, D, H, W = fmap1.shape
    assert (B, D, H, W) == (2, 256, 64, 64) and int(radius) == 4
    f1 = (fmap1.reshape(B, D, H * W) / np.float32(16.0)).astype(np.float32)
    f2 = fmap2.reshape(B, D, H * W).astype(np.float32)
    cx = coords[:, 0].reshape(B, H * W).astype(np.float32)
    cy = coords[:, 1].reshape(B, H * W).astype(np.float32)

    cores = []
    for c in range(NCORES):
        b, ps = c // 4, (c % 4) * NPC
        ccx = cx[b, ps : ps + NPC]
        ccy = cy[b, ps : ps + NPC]
        y0 = np.floor(ccy).astype(np.int64)
        order = np.argsort(y0, kind="stable")
        cores.append((b, ps, ccx, ccy, y0, order))

    # uniform (cross-core) per-block table bases and row counts
    tbases, rows = [], []
    for blk in range(NBLK):
        ylo = min(
            int(co[4][co[5][blk * 128 : (blk + 1) * 128]].min()) for co in cores
        )
        yhi = max(
            int(co[4][co[5][blk * 128 : (blk + 1) * 128]].max()) for co in cores
        )
        tbases.append(ylo - 4)
        rows.append(yhi + 6 - (ylo - 4))
    RP = max(rows)

    # max within-16-group y0 offset -> number of y taps
    omax = 0
    for co in cores:
        ys = co[4][co[5]]
        for g in range(NPC // 16):
            gy = ys[g * 16 : (g + 1) * 16]
            omax = max(omax, int(gy.max() - gy.min()))
    assert omax <= 6, f"group span {omax} too large for 16-row gather"
    YT = omax + 2
    assert YT <= 4, f"YT={YT}: widen scal layout for more y taps"
    params = (tuple(tbases), tuple(rows), RP, YT)

    in_maps = []
    for (b, ps, ccx, ccy, y0, order) in cores:
        f1c = np.ascontiguousarray(
            f1[b, :, ps : ps + NPC][:, order].reshape(2, 128, NPC).transpose(1, 0, 2)
        ).astype(bf16)
        f2c = np.ascontiguousarray(
            f2[b].reshape(2, 128, 4096).transpose(1, 0, 2)
        ).astype(bf16)

        ys = y0[order]
        us = (ccy - np.floor(ccy))[order].astype(np.float32)
        x0 = np.floor(ccx).astype(np.int64)[order]
        vs = (ccx - np.floor(ccx))[order].astype(np.float32)

        idxc = np.zeros((128, NBLK), np.int16)
        scalc = np.zeros((128, NBLK, 16), np.float32)
        for blk in range(NBLK):
            sl = slice(blk * 128, (blk + 1) * 128)
            yb, ub, xb, vb = ys[sl], us[sl], x0[sl], vs[sl]
            tb = tbases[blk]
            p = np.arange(128)
            grp = p // 16
            sg = np.array(
                [int(yb[g * 16 : (g + 1) * 16].min()) - 4 - tb for g in range(8)]
            )
            assert (sg >= 0).all() and (sg <= RP - 10).all()
            idxc[:, blk] = (sg[grp] + (p % 16)).astype(np.int16)
            o = (yb - 4 - tb) - sg[grp]          # within-group offset
            assert (o >= 0).all() and (o <= YT - 2).all()
            # y taps: weight (1-u) at tap o, u at tap o+1
            for t in range(YT):
                scalc[:, blk, t] = np.where(o == t, 1.0 - ub, 0.0) + np.where(
                    o + 1 == t, ub, 0.0
                )
            # x taps: weight (1-v) at tap s, v at tap s+1 (s = x0 & 1)
            s = (xb & 1).astype(np.int64)
            for t in range(3):
                scalc[:, blk, 4 + t] = np.where(s == t, 1.0 - vb, 0.0) + np.where(
                    s + 1 == t, vb, 0.0
                )
            # ladder: shift by (x0 >> 1) in int32 elems; binary bits of hsh
            hsh = xb >> 1
            for kst, shv in enumerate(XSH):
                scalc[:, blk, 8 + kst] = ((hsh // shv) % 2).astype(np.float32)
        in_maps.append(
            {
                "f1": f1c,
                "f2": f2c,
                "idx": np.ascontiguousarray(idxc),
                "scal": np.ascontiguousarray(scalc),
            }
        )
    return in_maps, params, [co[5] for co in cores]


def assemble(outs, orders):
    """8x [1024, 81] (sorted order; 81 = ytap*9+xtap) -> [2, 81, 64, 64].

    Reference quirk (RAFT): output tap index k = k1*9+k2 where k1 is the X
    offset and k2 the Y offset, so the x-tap axis goes in front.
    """
    full = np.zeros((NCORES, NPC, 81), np.float32)
    for i, (o, orderv) in enumerate(zip(outs, orders)):
        full[i][orderv] = np.asarray(o, dtype=np.float32)
    o = full.reshape(2, 4096, 81).reshape(2, 64, 64, 9, 9)
    return np.ascontiguousarray(
        o.transpose(0, 4, 3, 1, 2).reshape(2, 81, 64, 64)
    ).astype(np.float32)


def kernel(**inputs):
    fmap1 = np.asarray(inputs["fmap1"], np.float32)
    fmap2 = np.asarray(inputs["fmap2"], np.float32)
    coords = np.asarray(inputs["coords"], np.float32)
    radius = int(np.asarray(inputs["radius"]))
    in_maps, params, orders = host_prep(fmap1, fmap2, coords, radius)
    nc = get_nc(params)
    res = run_bass_kernel_spmd(nc, in_maps, core_ids=list(range(NCORES)))
    return assemble([r["out"] for r in res.results], orders)


# revision 15
# speedup vs baseline: 2.9292x; 1.1264x over previous
"""Trainium2 Bass kernel: nn_CorrBlockSingleScale (RAFT single-scale corr lookup).

reference: corr[n, m] = fmap1[b,:,n] . fmap2[b,:,m] / 16 over m = (ym, xm) in a
64x64 grid; out[b, k1*9+k2, h, w] = bilinear(corr[(h,w)], x=cx+d[k1], y=cy+d[k2]),
zeros padding (grid_sample align_corners=True, padding_mode='zeros').

Sharding: data-parallel over the B*H*W = 8192 pixel axis; core c handles batch
c//4, pixels (c%4)*1024 .. +1024.  No cross-core comms.

Pixels are HOST-SORTED by y0 = floor(cy).  A block of 128 sorted pixels spans
~9 distinct y0 values, so its correlation slice needs only ~20 rows of the
64-row (ym) grid:

  PE    : C[128pix, rows*64] = f1_blk^T @ f2[:, ...]        (bf16, k-outer)
  ACT   : PSUM -> ct rows (bf16), written into x-padded 74-wide rows
  Pool  : ap_gather (SBUF, per-16-partition-group start) pulls 16 padded rows
          covering every pixel's 10-row y-window
  DVE   : 4-stage in-place binary x-shift ladder (int32 pairs, shifts 32..4),
          then 5-tap x-lerp and (omax+2)-tap y-lerp, each as ONE broadcast
          tensor_tensor multiply + ONE innermost-axis tensor_reduce; residual
          x shift (x0 & 3) and within-group y offset are folded into the
          per-partition host-computed tap weights
Host: sort pixels, compute indices/weights, inverse-permute output.
"""

import numpy as np

import concourse.bass as bass
import concourse.mybir as mybir
import concourse.tile as tile
from concourse import bacc, library_config
from concourse.bass_utils import run_bass_kernel_spmd

F32 = mybir.dt.float32
BF16 = mybir.dt.bfloat16
I32 = mybir.dt.int32
I16 = mybir.dt.int16
COPY = mybir.ActivationFunctionType.Copy
MULT = mybir.AluOpType.mult
ADD = mybir.AluOpType.add
AXX = mybir.AxisListType.X

NCORES = 8
NPC = 1024          # pixels per core
NBLK = 8            # blocks of 128 sorted pixels
JUNK = 6            # extra ct rows so a 16-row gather may overrun data rows
WROW = 74           # padded ct row width (4 | 64 | 6), bf16
XSH = [16, 8, 4, 2]  # x ladder shifts in int32 elems (bf16: 32,16,8,4)
XT = 5              # x taps (residual shift x0 & 3 folded into weights)

_NC_CACHE = {}


def _s(st, b, j):
    """[128,1] per-partition scalar view of scal column j for block b."""
    return st[:, b : b + 1, j : j + 1].rearrange("p a c -> p (a c)")


def _taps(base, nrow, k, ntap, tstride=1):
    """[128, nrow, k, ntap] overlapped view: elem (r, c, t) = base[r, c + t]
    (tstride=1) or base[r + t, c] (tstride=row stride)."""
    a = base.ap
    return bass.AP(
        base.tensor,
        base.offset,
        [list(a[0]), list(a[1]), [1, k], [tstride, ntap]],
    )


def _wbc(st, b, j0, ntap, d1, d2):
    """Broadcast [128, d1, d2, ntap] view of scal cols j0..j0+ntap."""
    w = st[:, b : b + 1, j0 : j0 + ntap].rearrange("p a c -> p (a c)")
    return w.unsqueeze(1).unsqueeze(1).to_broadcast([128, d1, d2, ntap])


def _build_kernel(tc, out, f1, f2, idx, scal, tbases, rows, yts, RP):
    nc = tc.nc
    import contextlib

    RPA = RP + JUNK
    with contextlib.ExitStack() as ctx:
        const = ctx.enter_context(tc.tile_pool(name="const", bufs=1))
        work = ctx.enter_context(tc.tile_pool(name="work", bufs=3))
        cpool = ctx.enter_context(tc.tile_pool(name="cpool", bufs=2))
        psum = ctx.enter_context(tc.tile_pool(name="psum", bufs=2, space="PSUM"))

        nc.gpsimd.load_library(library_config.ap_gather)

        f1t = const.tile([128, 2, NPC], BF16)
        nc.sync.dma_start(f1t[:], f1[:])
        f2t = const.tile([128, 2, 4096], BF16)
        for cch in range(8):
            sl = slice(cch * 512, (cch + 1) * 512)
            nc.sync.dma_start(f2t[:, :, sl], f2[:, :, sl])
        idxt = const.tile([128, NBLK], I16)
        nc.sync.dma_start(idxt[:], idx[:])
        st = const.tile([128, NBLK, 16], F32)
        nc.sync.dma_start(st[:], scal[:])

        for b in range(NBLK):
            tbase = tbases[b]
            rws = rows[b]
            YT = yts[b]
            vlo = max(0, -tbase)            # first valid ct row
            vhi = min(rws, 64 - tbase)      # one past last valid ct row
            nval = (vhi - vlo) * 64
            c0 = (tbase + vlo) * 64         # f2 column of first valid row

            # ---- matmul: C_valid = f1_blk^T @ f2[:, c0 : c0+nval]  (k-outer)
            ps = psum.tile([128, RP * 64], F32, tag="ps")
            chunks = []
            o = 0
            while o < nval:
                n = min(512, nval - o)
                chunks.append((o, n))
                o += n
            for k in range(2):
                lhs = f1t[:, k : k + 1, b * 128 : (b + 1) * 128].rearrange(
                    "p a c -> p (a c)"
                )
                for (o, n) in chunks:
                    nc.tensor.matmul(
                        ps[:, o : o + n],
                        lhsT=lhs,
                        rhs=f2t[:, k : k + 1, c0 + o : c0 + o + n].rearrange(
                            "p a c -> p (a c)"
                        ),
                        start=(k == 0),
                        stop=(k == 1),
                    )

            # ---- PSUM -> ct (bf16) into x-padded 74-wide rows
            ct = cpool.tile([128, RPA, WROW], BF16, tag="C")
            if b < 2:
                # zero everything once per pool buffer: x pads stay zero
                # forever; junk/edge rows start zero (later blocks overwrite
                # data rows only, leaving finite values elsewhere)
                nc.gpsimd.memset(ct[:], 0.0)
            if vlo > 0:
                nc.gpsimd.memset(ct[:, 0:vlo, 4:68], 0.0)
            if vhi < rws:
                nc.gpsimd.memset(ct[:, vhi:rws, 4:68], 0.0)
            psv = ps[:, 0:nval].rearrange("p (r c) -> p r c", c=64)
            nc.scalar.copy(ct[:, vlo:vhi, 4:68], psv[:])

            # ---- SBUF gather: 16 padded rows per 16-partition group
            gp = work.tile([128, 16, WROW], BF16, tag="G")
            nc.gpsimd.ap_gather(
                gp[:].bitcast(I32),
                ct[:].bitcast(I32),
                idxt[:, b : b + 1],
                128,        # channels
                RPA,        # num_elems
                WROW // 2,  # d (int32 per padded row)
                16,         # num_idxs
            )

            # ---- in-place binary x-shift ladder on rows 0..12 (int32 view)
            g32 = gp[:, 0:12, :].bitcast(I32)   # [128, 12, 37]
            w = 37
            for kst, sh in enumerate(XSH):
                wn = w - sh
                mask = (
                    _s(st, b, 9 + kst).bitcast(I32).to_broadcast([128, 12, wn])
                )
                nc.vector.copy_predicated(
                    g32[:, :, 0:wn], mask, g32[:, :, sh : sh + wn]
                )
                w = wn
            gb = g32.bitcast(BF16)              # [128, 12, 14]

            # ---- 5-tap x-lerp: xo[r, k] = sum_t wv_t * gb[r, k + t]
            xprod = work.tile([128, 12, 9, XT], BF16, tag="XP")
            nc.vector.tensor_tensor(
                xprod[:],
                _taps(gb[:, :, 0:14], 12, 9, XT),
                _wbc(st, b, 4, XT, 12, 9),
                MULT,
            )
            xo = work.tile([128, 12, 9], F32, tag="XO")
            nc.vector.tensor_reduce(xo[:], xprod[:], AXX, ADD)

            # ---- YT-tap y-lerp: ot[j, k] = sum_t w_t * xo[j + t, k]
            yprod = work.tile([128, 9, 9, YT], BF16, tag="YP")
            nc.vector.tensor_tensor(
                yprod[:],
                _taps(xo[:, 0:9, :], 9, 9, YT, tstride=9),
                _wbc(st, b, 0, YT, 9, 9),
                MULT,
            )
            ot = work.tile([128, 9, 9], F32, tag="OT")
            nc.vector.tensor_reduce(ot[:], yprod[:], AXX, ADD)
            nc.sync.dma_start(
                out[b * 128 : (b + 1) * 128, :],
                ot[:].rearrange("p a c -> p (a c)"),
            )


def _build(params):
    tbases, rows, yts, RP = params
    nc = bacc.Bacc("TRN2", target_bir_lowering=False, debug=False)
    f1 = nc.dram_tensor("f1", [128, 2, NPC], BF16, kind="ExternalInput").ap()
    f2 = nc.dram_tensor("f2", [128, 2, 4096], BF16, kind="ExternalInput").ap()
    idx = nc.dram_tensor("idx", [128, NBLK], I16, kind="ExternalInput").ap()
    scal = nc.dram_tensor("scal", [128, NBLK, 16], F32, kind="ExternalInput").ap()
    out = nc.dram_tensor("out", [NPC, 81], F32, kind="ExternalOutput").ap()
    with tile.TileContext(nc) as tc:
        _build_kernel(tc, out, f1, f2, idx, scal, tbases, rows, yts, RP)
    nc.compile()
    return nc


def get_nc(params):
    if params not in _NC_CACHE:
        _NC_CACHE[params] = _build(params)
    return _NC_CACHE[params]


def host_prep(fmap1, fmap2, coords, radius):
    """Per-core input maps + compile params. All coord-derived logic here."""
    import ml_dtypes

    bf16 = ml_dtypes.bfloat16
    B, D, H, W = fmap1.shape
    assert (B, D, H, W) == (2, 256, 64, 64) and int(radius) == 4
    f1 = (fmap1.reshape(B, D, H * W) / np.float32(16.0)).astype(np.float32)
    f2 = fmap2.reshape(B, D, H * W).astype(np.float32)
    cx = coords[:, 0].reshape(B, H * W).astype(np.float32)
    cy = coords[:, 1].reshape(B, H * W).astype(np.float32)

    cores = []
    for c in range(NCORES):
        b, ps = c // 4, (c % 4) * NPC
        ccx = cx[b, ps : ps + NPC]
        ccy = cy[b, ps : ps + NPC]
        y0 = np.floor(ccy).astype(np.int64)
        order = np.argsort(y0, kind="stable")
        cores.append((b, ps, ccx, ccy, y0, order))

    # uniform (cross-core) per-block table bases, row counts, y-tap counts
    tbases, rows, yts = [], [], []
    for blk in range(NBLK):
        ylo = min(
            int(co[4][co[5][blk * 128 : (blk + 1) * 128]].min()) for co in cores
        )
        yhi = max(
            int(co[4][co[5][blk * 128 : (blk + 1) * 128]].max()) for co in cores
        )
        tbases.append(ylo - 4)
        rows.append(yhi + 6 - (ylo - 4))
        om = 0
        for co in cores:
            ys = co[4][co[5][blk * 128 : (blk + 1) * 128]]
            for g in range(8):
                gy = ys[g * 16 : (g + 1) * 16]
                om = max(om, int(gy.max() - gy.min()))
        assert om <= 2, f"block {blk}: group span {om} > 2"
        yts.append(om + 2)
    RP = max(rows)
    params = (tuple(tbases), tuple(rows), tuple(yts), RP)

    in_maps = []
    for (b, ps, ccx, ccy, y0, order) in cores:
        f1c = np.ascontiguousarray(
            f1[b, :, ps : ps + NPC][:, order].reshape(2, 128, NPC).transpose(1, 0, 2)
        ).astype(bf16)
        f2c = np.ascontiguousarray(
            f2[b].reshape(2, 128, 4096).transpose(1, 0, 2)
        ).astype(bf16)

        ys = y0[order]
        us = (ccy - np.floor(ccy))[order].astype(np.float32)
        x0 = np.floor(ccx).astype(np.int64)[order]
        vs = (ccx - np.floor(ccx))[order].astype(np.float32)

        idxc = np.zeros((128, NBLK), np.int16)
        scalc = np.zeros((128, NBLK, 16), np.float32)
        for blk in range(NBLK):
            sl = slice(blk * 128, (blk + 1) * 128)
            yb, ub, xb, vb = ys[sl], us[sl], x0[sl], vs[sl]
            tb = tbases[blk]
            p = np.arange(128)
            grp = p // 16
            sg = np.array(
                [int(yb[g * 16 : (g + 1) * 16].min()) - 4 - tb for g in range(8)]
            )
            assert (sg >= 0).all() and (sg <= rows[blk] - 10).all()
            idxc[:, blk] = (sg[grp] + (p % 16)).astype(np.int16)
            o = (yb - 4 - tb) - sg[grp]          # within-group offset
            assert (o >= 0).all() and (o <= yts[blk] - 2).all()
            # y taps: weight (1-u) at tap o, u at tap o+1
            for t in range(yts[blk]):
                scalc[:, blk, t] = np.where(o == t, 1.0 - ub, 0.0) + np.where(
                    o + 1 == t, ub, 0.0
                )
            # x taps: weight (1-v) at tap s, v at tap s+1 (s = x0 & 3)
            s = (xb & 3).astype(np.int64)
            for t in range(XT):
                scalc[:, blk, 4 + t] = np.where(s == t, 1.0 - vb, 0.0) + np.where(
                    s + 1 == t, vb, 0.0
                )
            # ladder: shift by 4*(x0 >> 2) bf16; binary bits of (x0 >> 2)
            hsh = xb >> 2
            for kst, shv in enumerate(XSH):
                scalc[:, blk, 9 + kst] = ((hsh // (shv // 2)) % 2).astype(
                    np.float32
                )
        in_maps.append(
            {
                "f1": f1c,
                "f2": f2c,
                "idx": np.ascontiguousarray(idxc),
                "scal": np.ascontiguousarray(scalc),
            }
        )
    return in_maps, params, [co[5] for co in cores]


def assemble(outs, orders):
    """8x [1024, 81] (sorted order; 81 = ytap*9+xtap) -> [2, 81, 64, 64].

    Reference quirk (RAFT): output tap index k = k1*9+k2 where k1 is the X
    offset and k2 the Y offset, so the x-tap axis goes in front.
    """
    full = np.zeros((NCORES, NPC, 81), np.float32)
    for i, (o, orderv) in enumerate(zip(outs, orders)):
        full[i][orderv] = np.asarray(o, dtype=np.float32)
    o = full.reshape(2, 4096, 81).reshape(2, 64, 64, 9, 9)
    return np.ascontiguousarray(
        o.transpose(0, 4, 3, 1, 2).reshape(2, 81, 64, 64)
    ).astype(np.float32)


def kernel(**inputs):
    fmap1 = np.asarray(inputs["fmap1"], np.float32)
    fmap2 = np.asarray(inputs["fmap2"], np.float32)
    coords = np.asarray(inputs["coords"], np.float32)
    radius = int(np.asarray(inputs["radius"]))
    in_maps, params, orders = host_prep(fmap1, fmap2, coords, radius)
    nc = get_nc(params)
    res = run_bass_kernel_spmd(nc, in_maps, core_ids=list(range(NCORES)))
    return assemble([r["out"] for r in res.results], orders)


# revision 21
# speedup vs baseline: 3.1887x; 1.0886x over previous
"""Trainium2 Bass kernel: nn_CorrBlockSingleScale (RAFT single-scale corr lookup).

reference: corr[n, m] = fmap1[b,:,n] . fmap2[b,:,m] / 16 over m = (ym, xm) in a
64x64 grid; out[b, k1*9+k2, h, w] = bilinear(corr[(h,w)], x=cx+d[k1], y=cy+d[k2]),
zeros padding (grid_sample align_corners=True, padding_mode='zeros').

Sharding: data-parallel over the B*H*W = 8192 pixel axis; core c handles batch
c//4, pixels (c%4)*1024 .. +1024.  No cross-core comms.

Pixels are HOST-SORTED by y0 = floor(cy).  A block of 128 sorted pixels spans
~9 distinct y0 values, so its correlation slice needs only ~20 rows of the
64-row (ym) grid:

  PE    : C[128pix, rows*64] = f1_blk^T @ f2[:, ...]        (bf16, k-outer)
  ACT   : PSUM -> ct rows (bf16), written into x-padded 74-wide rows
  Pool  : ap_gather (SBUF, per-16-partition-group start) pulls 16 padded rows
          covering every pixel's 10-row y-window
  DVE   : 4-stage in-place binary x-shift ladder (int32 pairs, shifts 32..4),
          then 5-tap x-lerp and (omax+2)-tap y-lerp, each as ONE broadcast
          tensor_tensor multiply + ONE innermost-axis tensor_reduce; residual
          x shift (x0 & 3) and within-group y offset are folded into the
          per-partition host-computed tap weights
Host: sort pixels, compute indices/weights, inverse-permute output.
"""

import numpy as np

import concourse.bass as bass
import concourse.mybir as mybir
import concourse.tile as tile
from concourse import bacc, library_config
from concourse.bass_utils import run_bass_kernel_spmd

F32 = mybir.dt.float32
BF16 = mybir.dt.bfloat16
I32 = mybir.dt.int32
I16 = mybir.dt.int16
COPY = mybir.ActivationFunctionType.Copy
MULT = mybir.AluOpType.mult
ADD = mybir.AluOpType.add
AXX = mybir.AxisListType.X

NCORES = 8
NPC = 1024          # pixels per core
NBLK = 8            # blocks of 128 sorted pixels
JUNK = 6            # extra ct rows so a 16-row gather may overrun data rows
WROW = 74           # padded ct row width (4 | 64 | 6), bf16
XSH = [16, 8, 4, 2, 1]  # x ladder shifts in int32 elems (bf16: 32,16,8,4,2)
XT = 3              # x taps (residual shift x0 & 1 folded into weights)

_NC_CACHE = {}


def _s(st, b, j):
    """[128,1] per-partition scalar view of scal column j for block b."""
    return st[:, b : b + 1, j : j + 1].rearrange("p a c -> p (a c)")


def _taps(base, nrow, k, ntap, tstride=1):
    """[128, nrow, k, ntap] overlapped view: elem (r, c, t) = base[r, c + t]
    (tstride=1) or base[r + t, c] (tstride=row stride)."""
    a = base.ap
    return bass.AP(
        base.tensor,
        base.offset,
        [list(a[0]), list(a[1]), [1, k], [tstride, ntap]],
    )


def _wbc(st, b, j0, ntap, d1, d2):
    """Broadcast [128, d1, d2, ntap] view of scal cols j0..j0+ntap."""
    w = st[:, b : b + 1, j0 : j0 + ntap].rearrange("p a c -> p (a c)")
    return w.unsqueeze(1).unsqueeze(1).to_broadcast([128, d1, d2, ntap])


def _build_kernel(tc, out, f1, f2, idx, scal, tbases, rows, yts, RP):
    nc = tc.nc
    import contextlib

    RPA = RP + JUNK
    with contextlib.ExitStack() as ctx:
        const = ctx.enter_context(tc.tile_pool(name="const", bufs=1))
        work = ctx.enter_context(tc.tile_pool(name="work", bufs=3))
        cpool = ctx.enter_context(tc.tile_pool(name="cpool", bufs=2))
        psum = ctx.enter_context(tc.tile_pool(name="psum", bufs=2, space="PSUM"))

        nc.gpsimd.load_library(library_config.ap_gather)

        f1t = const.tile([128, 2, NPC], BF16)
        nc.sync.dma_start(f1t[:], f1[:])
        f2t = const.tile([128, 2, 4096], BF16)
        for cch in range(8):
            sl = slice(cch * 512, (cch + 1) * 512)
            nc.sync.dma_start(f2t[:, :, sl], f2[:, :, sl])
        idxt = const.tile([128, NBLK], I16)
        nc.sync.dma_start(idxt[:], idx[:])
        st = const.tile([128, NBLK, 16], F32)
        nc.sync.dma_start(st[:], scal[:])

        for b in range(NBLK):
            tbase = tbases[b]
            rws = rows[b]
            YT = yts[b]
            vlo = max(0, -tbase)            # first valid ct row
            vhi = min(rws, 64 - tbase)      # one past last valid ct row
            nval = (vhi - vlo) * 64
            c0 = (tbase + vlo) * 64         # f2 column of first valid row

            # ---- matmul: C_valid = f1_blk^T @ f2[:, c0 : c0+nval]  (k-outer;
            # walrus caps matmul free size at 512 = one PSUM bank)
            ps = psum.tile([128, RP * 64], F32, tag="ps")
            chunks = []
            o = 0
            while o < nval:
                n = min(512, nval - o)
                chunks.append((o, n))
                o += n
            for k in range(2):
                lhs = f1t[:, k : k + 1, b * 128 : (b + 1) * 128].rearrange(
                    "p a c -> p (a c)"
                )
                for (o, n) in chunks:
                    nc.tensor.matmul(
                        ps[:, o : o + n],
                        lhsT=lhs,
                        rhs=f2t[:, k : k + 1, c0 + o : c0 + o + n].rearrange(
                            "p a c -> p (a c)"
                        ),
                        start=(k == 0),
                        stop=(k == 1),
                    )

            # ---- PSUM -> ct (bf16) into x-padded 74-wide rows
            ct = cpool.tile([128, RPA, WROW], BF16, tag="C")
            if b < 2:
                # zero everything once per pool buffer: x pads stay zero
                # forever; junk/edge rows start zero (later blocks overwrite
                # data rows only, leaving finite values elsewhere). DVE is
                # idle at kernel start, so do these there.
                nc.vector.memset(ct[:], 0.0)
            if vlo > 0:
                nc.gpsimd.memset(ct[:, 0:vlo, 4:68], 0.0)
            if vhi < rws:
                nc.gpsimd.memset(ct[:, vhi:rws, 4:68], 0.0)
            psv = ps[:, 0:nval].rearrange("p (r c) -> p r c", c=64)
            nc.scalar.copy(ct[:, vlo:vhi, 4:68], psv[:])

            # ---- SBUF gather: 16 padded rows per 16-partition group
            gp = work.tile([128, 16, WROW], BF16, tag="G")
            nc.gpsimd.ap_gather(
                gp[:].bitcast(I32),
                ct[:].bitcast(I32),
                idxt[:, b : b + 1],
                128,        # channels
                RPA,        # num_elems
                WROW // 2,  # d (int32 per padded row)
                16,         # num_idxs
            )

            # ---- in-place binary x-shift ladder on rows 0..12 (int32 view)
            nrw = 9 + YT - 1                    # gather rows the taps touch
            g32 = gp[:, 0:nrw, :].bitcast(I32)  # [128, nrw, 37]
            w = 37
            for kst, sh in enumerate(XSH):
                wn = w - sh
                mask = (
                    _s(st, b, 8 + kst).bitcast(I32).to_broadcast([128, nrw, wn])
                )
                nc.vector.copy_predicated(
                    g32[:, :, 0:wn], mask, g32[:, :, sh : sh + wn]
                )
                w = wn
            gb = g32.bitcast(BF16)              # [128, nrw, 12]

            # ---- 3-tap x-lerp: xo[r, k] = sum_t wv_t * gb[r, k + t]
            xo = work.tile([128, 12, 9], BF16, tag="XO")
            xov = xo[:, 0:nrw, :]
            nc.scalar.activation(xov, gb[:, :, 0:9], COPY, scale=_s(st, b, 4))
            for t in (1, 2):
                nc.vector.scalar_tensor_tensor(
                    xov, gb[:, :, t : t + 9], _s(st, b, 4 + t), xov, MULT, ADD
                )

            # ---- YT-tap y-lerp: ot[j, k] = sum_t w_t * xo[j + t, k]
            ot = work.tile([128, 9, 9], F32, tag="OT")
            nc.scalar.activation(ot[:], xo[:, 0:9, :], COPY, scale=_s(st, b, 0))
            for t in range(1, YT):
                nc.vector.scalar_tensor_tensor(
                    ot[:], xo[:, t : t + 9, :], _s(st, b, t), ot[:], MULT, ADD
                )
            nc.sync.dma_start(
                out[b * 128 : (b + 1) * 128, :],
                ot[:].rearrange("p a c -> p (a c)"),
            )


def _build(params):
    tbases, rows, yts, RP = params
    nc = bacc.Bacc("TRN2", target_bir_lowering=False, debug=False)
    f1 = nc.dram_tensor("f1", [128, 2, NPC], BF16, kind="ExternalInput").ap()
    f2 = nc.dram_tensor("f2", [128, 2, 4096], BF16, kind="ExternalInput").ap()
    idx = nc.dram_tensor("idx", [128, NBLK], I16, kind="ExternalInput").ap()
    scal = nc.dram_tensor("scal", [128, NBLK, 16], F32, kind="ExternalInput").ap()
    out = nc.dram_tensor("out", [NPC, 81], F32, kind="ExternalOutput").ap()
    with tile.TileContext(nc) as tc:
        _build_kernel(tc, out, f1, f2, idx, scal, tbases, rows, yts, RP)
    nc.compile()
    return nc


def get_nc(params):
    if params not in _NC_CACHE:
        _NC_CACHE[params] = _build(params)
    return _NC_CACHE[params]


def host_prep(fmap1, fmap2, coords, radius):
    """Per-core input maps + compile params. All coord-derived logic here."""
    import ml_dtypes

    bf16 = ml_dtypes.bfloat16
    B, D, H, W = fmap1.shape
    assert (B, D, H, W) == (2, 256, 64, 64) and int(radius) == 4
    f1 = (fmap1.reshape(B, D, H * W) / np.float32(16.0)).astype(np.float32)
    f2 = fmap2.reshape(B, D, H * W).astype(np.float32)
    cx = coords[:, 0].reshape(B, H * W).astype(np.float32)
    cy = coords[:, 1].reshape(B, H * W).astype(np.float32)

    cores = []
    for c in range(NCORES):
        b, ps = c // 4, (c % 4) * NPC
        ccx = cx[b, ps : ps + NPC]
        ccy = cy[b, ps : ps + NPC]
        y0 = np.floor(ccy).astype(np.int64)
        order = np.argsort(y0, kind="stable")
        cores.append((b, ps, ccx, ccy, y0, order))

    # uniform (cross-core) per-block table bases, row counts, y-tap counts
    tbases, rows, yts = [], [], []
    for blk in range(NBLK):
        ylo = min(
            int(co[4][co[5][blk * 128 : (blk + 1) * 128]].min()) for co in cores
        )
        yhi = max(
            int(co[4][co[5][blk * 128 : (blk + 1) * 128]].max()) for co in cores
        )
        tbases.append(ylo - 4)
        rows.append(yhi + 6 - (ylo - 4))
        om = 0
        for co in cores:
            ys = co[4][co[5][blk * 128 : (blk + 1) * 128]]
            for g in range(8):
                gy = ys[g * 16 : (g + 1) * 16]
                om = max(om, int(gy.max() - gy.min()))
        assert om <= 2, f"block {blk}: group span {om} > 2"
        yts.append(om + 2)
    RP = max(rows)
    params = (tuple(tbases), tuple(rows), tuple(yts), RP)

    in_maps = []
    for (b, ps, ccx, ccy, y0, order) in cores:
        f1c = np.ascontiguousarray(
            f1[b, :, ps : ps + NPC][:, order].reshape(2, 128, NPC).transpose(1, 0, 2)
        ).astype(bf16)
        f2c = np.ascontiguousarray(
            f2[b].reshape(2, 128, 4096).transpose(1, 0, 2)
        ).astype(bf16)

        ys = y0[order]
        us = (ccy - np.floor(ccy))[order].astype(np.float32)
        x0 = np.floor(ccx).astype(np.int64)[order]
        vs = (ccx - np.floor(ccx))[order].astype(np.float32)

        idxc = np.zeros((128, NBLK), np.int16)
        scalc = np.zeros((128, NBLK, 16), np.float32)
        for blk in range(NBLK):
            sl = slice(blk * 128, (blk + 1) * 128)
            yb, ub, xb, vb = ys[sl], us[sl], x0[sl], vs[sl]
            tb = tbases[blk]
            p = np.arange(128)
            grp = p // 16
            sg = np.array(
                [int(yb[g * 16 : (g + 1) * 16].min()) - 4 - tb for g in range(8)]
            )
            assert (sg >= 0).all() and (sg <= rows[blk] - 10).all()
            idxc[:, blk] = (sg[grp] + (p % 16)).astype(np.int16)
            o = (yb - 4 - tb) - sg[grp]          # within-group offset
            assert (o >= 0).all() and (o <= yts[blk] - 2).all()
            # y taps: weight (1-u) at tap o, u at tap o+1
            for t in range(yts[blk]):
                scalc[:, blk, t] = np.where(o == t, 1.0 - ub, 0.0) + np.where(
                    o + 1 == t, ub, 0.0
                )
            # x taps: weight (1-v) at tap s, v at tap s+1 (s = x0 & 1)
            s = (xb & 1).astype(np.int64)
            for t in range(XT):
                scalc[:, blk, 4 + t] = np.where(s == t, 1.0 - vb, 0.0) + np.where(
                    s + 1 == t, vb, 0.0
                )
            # ladder: shift by 2*(x0 >> 1) bf16; binary bits of (x0 >> 1)
            hsh = xb >> 1
            for kst, shv in enumerate(XSH):
                scalc[:, blk, 8 + kst] = ((hsh // shv) % 2).astype(np.float32)
        in_maps.append(
            {
                "f1": f1c,
                "f2": f2c,
                "idx": np.ascontiguousarray(idxc),
                "scal": np.ascontiguousarray(scalc),
            }
        )
    return in_maps, params, [co[5] for co in cores]


def assemble(outs, orders):
    """8x [1024, 81] (sorted order; 81 = ytap*9+xtap) -> [2, 81, 64, 64].

    Reference quirk (RAFT): output tap index k = k1*9+k2 where k1 is the X
    offset and k2 the Y offset, so the x-tap axis goes in front.
    """
    full = np.zeros((NCORES, NPC, 81), np.float32)
    for i, (o, orderv) in enumerate(zip(outs, orders)):
        full[i][orderv] = np.asarray(o, dtype=np.float32)
    o = full.reshape(2, 4096, 81).reshape(2, 64, 64, 9, 9)
    return np.ascontiguousarray(
        o.transpose(0, 4, 3, 1, 2).reshape(2, 81, 64, 64)
    ).astype(np.float32)


def kernel(**inputs):
    fmap1 = np.asarray(inputs["fmap1"], np.float32)
    fmap2 = np.asarray(inputs["fmap2"], np.float32)
    coords = np.asarray(inputs["coords"], np.float32)
    radius = int(np.asarray(inputs["radius"]))
    in_maps, params, orders = host_prep(fmap1, fmap2, coords, radius)
    nc = get_nc(params)
    res = run_bass_kernel_spmd(nc, in_maps, core_ids=list(range(NCORES)))
    return assemble([r["out"] for r in res.results], orders)


# revision 25
# speedup vs baseline: 3.2740x; 1.0267x over previous
"""Trainium2 Bass kernel: nn_CorrBlockSingleScale (RAFT single-scale corr lookup).

reference: corr[n, m] = fmap1[b,:,n] . fmap2[b,:,m] / 16 over m = (ym, xm) in a
64x64 grid; out[b, k1*9+k2, h, w] = bilinear(corr[(h,w)], x=cx+d[k1], y=cy+d[k2]),
zeros padding (grid_sample align_corners=True, padding_mode='zeros').

Sharding: data-parallel over the B*H*W = 8192 pixel axis; core c handles batch
c//4, pixels (c%4)*1024 .. +1024.  No cross-core comms.

Pixels are HOST-SORTED by y0 = floor(cy).  A block of 128 sorted pixels spans
~9 distinct y0 values, so its correlation slice needs only ~20 rows of the
64-row (ym) grid:

  PE    : C[128pix, rows*64] = f1_blk^T @ f2[:, ...]        (bf16, k-outer)
  ACT   : PSUM -> ct rows (bf16), written into x-padded 74-wide rows
  Pool  : ap_gather (SBUF, per-16-partition-group start) pulls 16 padded rows
          covering every pixel's 10-row y-window
  DVE   : 4-stage in-place binary x-shift ladder (int32 pairs, shifts 32..4),
          then 5-tap x-lerp and (omax+2)-tap y-lerp, each as ONE broadcast
          tensor_tensor multiply + ONE innermost-axis tensor_reduce; residual
          x shift (x0 & 3) and within-group y offset are folded into the
          per-partition host-computed tap weights
Host: sort pixels, compute indices/weights, inverse-permute output.
"""

import numpy as np

import concourse.bass as bass
import concourse.mybir as mybir
import concourse.tile as tile
from concourse import bacc, library_config
from concourse.bass_utils import run_bass_kernel_spmd

F32 = mybir.dt.float32
BF16 = mybir.dt.bfloat16
I32 = mybir.dt.int32
I16 = mybir.dt.int16
COPY = mybir.ActivationFunctionType.Copy
MULT = mybir.AluOpType.mult
ADD = mybir.AluOpType.add
AXX = mybir.AxisListType.X

NCORES = 8
NPC = 1024          # pixels per core
NBLK = 8            # blocks of 128 sorted pixels
JUNK = 6            # extra ct rows so a 16-row gather may overrun data rows
WROW = 74           # padded ct row width (4 | 64 | 6), bf16
XSH = [16, 8, 4, 2, 1]  # x ladder shifts in int32 elems (bf16: 32,16,8,4,2)
XT = 3              # x taps (residual shift x0 & 1 folded into weights)

_NC_CACHE = {}


def _s(st, b, j):
    """[128,1] per-partition scalar view of scal column j for block b."""
    return st[:, b : b + 1, j : j + 1].rearrange("p a c -> p (a c)")


def _taps(base, nrow, k, ntap, tstride=1):
    """[128, nrow, k, ntap] overlapped view: elem (r, c, t) = base[r, c + t]
    (tstride=1) or base[r + t, c] (tstride=row stride)."""
    a = base.ap
    return bass.AP(
        base.tensor,
        base.offset,
        [list(a[0]), list(a[1]), [1, k], [tstride, ntap]],
    )


def _wbc(st, b, j0, ntap, d1, d2):
    """Broadcast [128, d1, d2, ntap] view of scal cols j0..j0+ntap."""
    w = st[:, b : b + 1, j0 : j0 + ntap].rearrange("p a c -> p (a c)")
    return w.unsqueeze(1).unsqueeze(1).to_broadcast([128, d1, d2, ntap])


def _build_kernel(tc, out, f1, f2, idx, scal, tbases, rows, yts, RP):
    nc = tc.nc
    import contextlib

    RPA = RP + JUNK
    with contextlib.ExitStack() as ctx:
        const = ctx.enter_context(tc.tile_pool(name="const", bufs=1))
        work = ctx.enter_context(tc.tile_pool(name="work", bufs=4))
        cpool = ctx.enter_context(tc.tile_pool(name="cpool", bufs=3))
        opool = ctx.enter_context(tc.tile_pool(name="opool", bufs=2))
        psum = ctx.enter_context(tc.tile_pool(name="psum", bufs=2, space="PSUM"))

        nc.gpsimd.load_library(library_config.ap_gather)

        # load order tuned so block 0's matmul can start ASAP: its lhs
        # (f1 block 0) and first rhs chunks go first
        f1t = const.tile([128, 2, NPC], BF16)
        nc.sync.dma_start(f1t[:, :, 0:128], f1[:, :, 0:128])
        f2t = const.tile([128, 2, 4096], BF16)
        nc.sync.dma_start(f2t[:, :, 0:512], f2[:, :, 0:512])
        nc.sync.dma_start(f2t[:, :, 512:1024], f2[:, :, 512:1024])
        nc.sync.dma_start(f1t[:, :, 128:NPC], f1[:, :, 128:NPC])
        idxt = const.tile([128, NBLK], I16)
        nc.sync.dma_start(idxt[:], idx[:])
        st = const.tile([128, NBLK, 16], F32)
        nc.sync.dma_start(st[:], scal[:])
        for cch in range(3):
            sl = slice(1024 + cch * 1024, 2048 + cch * 1024)
            nc.sync.dma_start(f2t[:, :, sl], f2[:, :, sl])

        for b in range(NBLK):
            tbase = tbases[b]
            rws = rows[b]
            YT = yts[b]
            vlo = max(0, -tbase)            # first valid ct row
            vhi = min(rws, 64 - tbase)      # one past last valid ct row
            nval = (vhi - vlo) * 64
            c0 = (tbase + vlo) * 64         # f2 column of first valid row

            # ---- matmul: C_valid = f1_blk^T @ f2[:, c0 : c0+nval]  (k-outer;
            # walrus caps matmul free size at 512 = one PSUM bank)
            ps = psum.tile([128, RP * 64], F32, tag="ps")
            chunks = []
            o = 0
            while o < nval:
                n = min(512, nval - o)
                chunks.append((o, n))
                o += n
            for k in range(2):
                lhs = f1t[:, k : k + 1, b * 128 : (b + 1) * 128].rearrange(
                    "p a c -> p (a c)"
                )
                for (o, n) in chunks:
                    nc.tensor.matmul(
                        ps[:, o : o + n],
                        lhsT=lhs,
                        rhs=f2t[:, k : k + 1, c0 + o : c0 + o + n].rearrange(
                            "p a c -> p (a c)"
                        ),
                        start=(k == 0),
                        stop=(k == 1),
                    )

            # ---- PSUM -> ct (bf16) into x-padded 74-wide rows
            ct = cpool.tile([128, RPA, WROW], BF16, tag="C")
            if b < 3:
                # zero everything once per pool buffer: x pads stay zero
                # forever; junk/edge rows start zero (later blocks overwrite
                # data rows only, leaving finite values elsewhere). DVE is
                # idle at kernel start, so do these there.
                nc.vector.memset(ct[:].bitcast(I32), 0.0)
            if vlo > 0:
                nc.gpsimd.memset(ct[:, 0:vlo, 4:68], 0.0)
            if vhi < rws:
                nc.gpsimd.memset(ct[:, vhi:rws, 4:68], 0.0)
            psv = ps[:, 0:nval].rearrange("p (r c) -> p r c", c=64)
            nc.scalar.copy(ct[:, vlo:vhi, 4:68], psv[:])

            # ---- SBUF gather: 16 padded rows per 16-partition group
            gp = work.tile([128, 16, WROW], BF16, tag="G")
            nc.gpsimd.ap_gather(
                gp[:].bitcast(I32),
                ct[:].bitcast(I32),
                idxt[:, b : b + 1],
                128,        # channels
                RPA,        # num_elems
                WROW // 2,  # d (int32 per padded row)
                16,         # num_idxs
            )

            # ---- in-place binary x-shift ladder on rows 0..12 (int32 view)
            nrw = 9 + YT - 1                    # gather rows the taps touch
            g32 = gp[:, 0:nrw, :].bitcast(I32)  # [128, nrw, 37]
            w = 37
            for kst, sh in enumerate(XSH):
                wn = w - sh
                mask = (
                    _s(st, b, 8 + kst).bitcast(I32).to_broadcast([128, nrw, wn])
                )
                nc.vector.copy_predicated(
                    g32[:, :, 0:wn], mask, g32[:, :, sh : sh + wn]
                )
                w = wn
            gb = g32.bitcast(BF16)              # [128, nrw, 12]

            # ---- 3-tap x-lerp: xo[r, k] = sum_t wv_t * gb[r, k + t]
            xo = work.tile([128, 12, 9], BF16, tag="XO")
            xov = xo[:, 0:nrw, :]
            nc.scalar.activation(xov, gb[:, :, 0:9], COPY, scale=_s(st, b, 4))
            for t in (1, 2):
                nc.vector.scalar_tensor_tensor(
                    xov, gb[:, :, t : t + 9], _s(st, b, 4 + t), xov, MULT, ADD
                )

            # ---- YT-tap y-lerp: ot[j, k] = sum_t w_t * xo[j + t, k]
            if b % 4 == 0:
                ob = opool.tile([128, 4, 81], F32, tag="OB")
            ot = ob[:, b % 4 : b % 4 + 1, :].rearrange(
                "p a (d c) -> p (a d) c", c=9
            )
            nc.scalar.activation(ot, xo[:, 0:9, :], COPY, scale=_s(st, b, 0))
            for t in range(1, YT):
                nc.vector.scalar_tensor_tensor(
                    ot, xo[:, t : t + 9, :], _s(st, b, t), ot, MULT, ADD
                )
            if b % 4 == 3:
                # one batched out-DMA per 4 blocks (fewer sync-queue instrs)
                nc.sync.dma_start(
                    out[(b - 3) * 128 : (b + 1) * 128, :].rearrange(
                        "(a p) c -> p a c", a=4
                    ),
                    ob[:],
                )


def _build(params):
    tbases, rows, yts, RP = params
    nc = bacc.Bacc("TRN2", target_bir_lowering=False, debug=False)
    f1 = nc.dram_tensor("f1", [128, 2, NPC], BF16, kind="ExternalInput").ap()
    f2 = nc.dram_tensor("f2", [128, 2, 4096], BF16, kind="ExternalInput").ap()
    idx = nc.dram_tensor("idx", [128, NBLK], I16, kind="ExternalInput").ap()
    scal = nc.dram_tensor("scal", [128, NBLK, 16], F32, kind="ExternalInput").ap()
    out = nc.dram_tensor("out", [NPC, 81], F32, kind="ExternalOutput").ap()
    with tile.TileContext(nc) as tc:
        _build_kernel(tc, out, f1, f2, idx, scal, tbases, rows, yts, RP)
    nc.compile()
    return nc


def get_nc(params):
    if params not in _NC_CACHE:
        _NC_CACHE[params] = _build(params)
    return _NC_CACHE[params]


def host_prep(fmap1, fmap2, coords, radius):
    """Per-core input maps + compile params. All coord-derived logic here."""
    import ml_dtypes

    bf16 = ml_dtypes.bfloat16
    B, D, H, W = fmap1.shape
    assert (B, D, H, W) == (2, 256, 64, 64) and int(radius) == 4
    f1 = (fmap1.reshape(B, D, H * W) / np.float32(16.0)).astype(np.float32)
    f2 = fmap2.reshape(B, D, H * W).astype(np.float32)
    cx = coords[:, 0].reshape(B, H * W).astype(np.float32)
    cy = coords[:, 1].reshape(B, H * W).astype(np.float32)

    cores = []
    for c in range(NCORES):
        b, ps = c // 4, (c % 4) * NPC
        ccx = cx[b, ps : ps + NPC]
        ccy = cy[b, ps : ps + NPC]
        y0 = np.floor(ccy).astype(np.int64)
        order = np.argsort(y0, kind="stable")
        cores.append((b, ps, ccx, ccy, y0, order))

    # uniform (cross-core) per-block table bases, row counts, y-tap counts
    tbases, rows, yts = [], [], []
    for blk in range(NBLK):
        ylo = min(
            int(co[4][co[5][blk * 128 : (blk + 1) * 128]].min()) for co in cores
        )
        yhi = max(
            int(co[4][co[5][blk * 128 : (blk + 1) * 128]].max()) for co in cores
        )
        tbases.append(ylo - 4)
        rows.append(yhi + 6 - (ylo - 4))
        om = 0
        for co in cores:
            ys = co[4][co[5][blk * 128 : (blk + 1) * 128]]
            for g in range(8):
                gy = ys[g * 16 : (g + 1) * 16]
                om = max(om, int(gy.max() - gy.min()))
        assert om <= 2, f"block {blk}: group span {om} > 2"
        yts.append(om + 2)
    RP = max(rows)
    params = (tuple(tbases), tuple(rows), tuple(yts), RP)

    in_maps = []
    for (b, ps, ccx, ccy, y0, order) in cores:
        f1c = np.ascontiguousarray(
            f1[b, :, ps : ps + NPC][:, order].reshape(2, 128, NPC).transpose(1, 0, 2)
        ).astype(bf16)
        f2c = np.ascontiguousarray(
            f2[b].reshape(2, 128, 4096).transpose(1, 0, 2)
        ).astype(bf16)

        ys = y0[order]
        us = (ccy - np.floor(ccy))[order].astype(np.float32)
        x0 = np.floor(ccx).astype(np.int64)[order]
        vs = (ccx - np.floor(ccx))[order].astype(np.float32)

        idxc = np.zeros((128, NBLK), np.int16)
        scalc = np.zeros((128, NBLK, 16), np.float32)
        for blk in range(NBLK):
            sl = slice(blk * 128, (blk + 1) * 128)
            yb, ub, xb, vb = ys[sl], us[sl], x0[sl], vs[sl]
            tb = tbases[blk]
            p = np.arange(128)
            grp = p // 16
            sg = np.array(
                [int(yb[g * 16 : (g + 1) * 16].min()) - 4 - tb for g in range(8)]
            )
            assert (sg >= 0).all() and (sg <= rows[blk] - 10).all()
            idxc[:, blk] = (sg[grp] + (p % 16)).astype(np.int16)
            o = (yb - 4 - tb) - sg[grp]          # within-group offset
            assert (o >= 0).all() and (o <= yts[blk] - 2).all()
            # y taps: weight (1-u) at tap o, u at tap o+1
            for t in range(yts[blk]):
                scalc[:, blk, t] = np.where(o == t, 1.0 - ub, 0.0) + np.where(
                    o + 1 == t, ub, 0.0
                )
            # x taps: weight (1-v) at tap s, v at tap s+1 (s = x0 & 1)
            s = (xb & 1).astype(np.int64)
            for t in range(XT):
                scalc[:, blk, 4 + t] = np.where(s == t, 1.0 - vb, 0.0) + np.where(
                    s + 1 == t, vb, 0.0
                )
            # ladder: shift by 2*(x0 >> 1) bf16; binary bits of (x0 >> 1)
            hsh = xb >> 1
            for kst, shv in enumerate(XSH):
                scalc[:, blk, 8 + kst] = ((hsh // shv) % 2).astype(np.float32)
        in_maps.append(
            {
                "f1": f1c,
                "f2": f2c,
                "idx": np.ascontiguousarray(idxc),
                "scal": np.ascontiguousarray(scalc),
            }
        )
    return in_maps, params, [co[5] for co in cores]


def assemble(outs, orders):
    """8x [1024, 81] (sorted order; 81 = ytap*9+xtap) -> [2, 81, 64, 64].

    Reference quirk (RAFT): output tap index k = k1*9+k2 where k1 is the X
    offset and k2 the Y offset, so the x-tap axis goes in front.
    """
    full = np.zeros((NCORES, NPC, 81), np.float32)
    for i, (o, orderv) in enumerate(zip(outs, orders)):
        full[i][orderv] = np.asarray(o, dtype=np.float32)
    o = full.reshape(2, 4096, 81).reshape(2, 64, 64, 9, 9)
    return np.ascontiguousarray(
        o.transpose(0, 4, 3, 1, 2).reshape(2, 81, 64, 64)
    ).astype(np.float32)


def kernel(**inputs):
    fmap1 = np.asarray(inputs["fmap1"], np.float32)
    fmap2 = np.asarray(inputs["fmap2"], np.float32)
    coords = np.asarray(inputs["coords"], np.float32)
    radius = int(np.asarray(inputs["radius"]))
    in_maps, params, orders = host_prep(fmap1, fmap2, coords, radius)
    nc = get_nc(params)
    res = run_bass_kernel_spmd(nc, in_maps, core_ids=list(range(NCORES)))
    return assemble([r["out"] for r in res.results], orders)
